# revision 4
# baseline (speedup 1.0000x reference)
# Trainium2 Bass kernel for nn_Conv2dSDK_QR: low-rank (Q @ R) factorized
# stride-1 3x3 conv expressed as two matmuls over 4x4/stride-2 windows.
#
# Math (per image, validated vs reference):
#   xp = zero-pad(x, 1)                              [128, 66, 66]
#   flatT[win*128+c, vi*32+vj] = xp[c, i+2vi, j+2vj] (win = i*4+j)
#   tT = R2 @ flatT                                  [256, 1024]
#   yT = Q @ tT                                      [512, 1024]
#   out[oc, 2vi+top, 2vj+left] = yT[(top*2+left)*128+oc, vi*32+vj]
# where R2 is R with columns permuted from (c*16+win) to (win*128+c)
# ordering, so each win-chunk of flatT is just a strided view of xp.
#
# Sharding: data-parallel over batch, 4 images per core across 8 cores.
# Weights (R2^T chunks, Q^T chunks) replicated.

import numpy as np

import concourse.bass as bass
import concourse.bacc as bacc
import concourse.mybir as mybir
import concourse.tile as tile
from concourse.bass_utils import run_bass_kernel_spmd

N_CORES = 8
N_PER_CORE = 4
C = 128          # channels (= partition dim)
H = W = 64
HP = WP = 66     # padded spatial
RANK = 256
MOUT = 512       # 4 placements * 128 out channels
NWIN = 16        # 4x4 window positions
DT = mybir.dt.float32


def build_nc(n_per_core=N_PER_CORE, mm_dtype=mybir.dt.float32r):
    nc = bacc.Bacc()
    x_ext = nc.declare_dram_parameter("x", [n_per_core, C, HP, WP], mm_dtype, isOutput=False)
    r_ext = nc.declare_dram_parameter("r2t", [C, NWIN, RANK], mm_dtype, isOutput=False)
    q_ext = nc.declare_dram_parameter("qt", [C, 2, MOUT], mm_dtype, isOutput=False)
    y_ext = nc.declare_dram_parameter("y", [n_per_core, C, H, W], DT, isOutput=True)

    with tile.TileContext(nc) as tc:
        with (
            tc.tile_pool(name="weights", bufs=1) as wpool,
            tc.tile_pool(name="xp", bufs=2) as xpool,
            tc.tile_pool(name="tt", bufs=2) as tpool,
            tc.tile_pool(name="osb", bufs=2) as opool,
            tc.tile_pool(name="pt", bufs=4, space="PSUM") as ptpool,
            tc.tile_pool(name="py", bufs=4, space="PSUM") as pypool,
        ):
            # r2t[c, win, r] = R2[r, win*128+c]  (lhsT chunks for matmul 1)
            r2t = wpool.tile([C, NWIN, RANK], mm_dtype)
            # qt[r_in, rc, m] = Q[m, rc*128+r_in]  (lhsT chunks for matmul 2)
            qt = wpool.tile([C, 2, MOUT], mm_dtype)
            nc.sync.dma_start(r2t[:], r_ext[:])
            nc.sync.dma_start(qt[:], q_ext[:])

            for n in range(n_per_core):
                xp = xpool.tile([C, HP, WP], mm_dtype)
                nc.sync.dma_start(xp[:], x_ext[n])
                # tT[r_in, rc, lb, vi_in, vj] = tT_full[rc*128+r_in, (16*lb+vi_in)*32+vj]
                tT = tpool.tile([C, 2, 2, 16, 32], mm_dtype)
                osb = opool.tile([C, H, W], DT)
                for lb in range(2):   # l-blocks of 512 positions (16 vi rows)
                    for rc in range(2):   # rank tiles of 128
                        pt = ptpool.tile([128, 16, 32], DT)
                        for win in range(NWIN):
                            i, j = divmod(win, 4)
                            rhs = xp[:, i + 32 * lb : i + 32 * lb + 31 : 2, j : j + 63 : 2]
                            nc.tensor.matmul(
                                pt[:],
                                r2t[:, win, rc * 128 : (rc + 1) * 128],
                                rhs,
                                start=(win == 0),
                                stop=(win == NWIN - 1),
                            )
                        nc.vector.tensor_copy(tT[:, rc, lb], pt[:])
                    for mt in range(4):   # output row tiles: m = mt*128 + oc
                        py = pypool.tile([128, 16, 32], DT)
                        for rc in range(2):
                            nc.tensor.matmul(
                                py[:],
                                qt[:, rc, mt * 128 : (mt + 1) * 128],
                                tT[:, rc, lb],
                                start=(rc == 0),
                                stop=(rc == 1),
                            )
                        top, left = divmod(mt, 2)
                        dest = osb[:, 32 * lb + top : 32 * lb + top + 31 : 2, left : left + 63 : 2]
                        nc.vector.tensor_copy(dest, py[:])
                nc.sync.dma_start(y_ext[n], osb[:])
    nc.finalize()
    return nc


def make_host_inputs(x, Q, R):
    """Full inputs -> (padded x, r2t, qt) host arrays."""
    x = np.asarray(x, dtype=np.float32)
    Q = np.asarray(Q, dtype=np.float32)
    R = np.asarray(R, dtype=np.float32)
    n = x.shape[0]
    xpad = np.zeros((n, C, HP, WP), np.float32)
    xpad[:, :, 1 : 1 + H, 1 : 1 + W] = x
    # permute R columns from (c*16+win) to (win*128+c)
    R2 = R.reshape(RANK, C, NWIN).transpose(0, 2, 1).reshape(RANK, C * NWIN)
    r2t = np.ascontiguousarray(R2.reshape(RANK, NWIN, C).transpose(2, 1, 0))
    qt = np.ascontiguousarray(Q.reshape(MOUT, 2, 128).transpose(2, 1, 0))
    return xpad, r2t, qt


_NC_CACHE = {}


def kernel(x, Q, R):
    xpad, r2t, qt = make_host_inputs(x, Q, R)
    n = xpad.shape[0]
    assert n == N_CORES * N_PER_CORE
    if "nc" not in _NC_CACHE:
        _NC_CACHE["nc"] = build_nc()
    nc = _NC_CACHE["nc"]
    in_maps = [
        {
            "x": np.ascontiguousarray(xpad[i * N_PER_CORE : (i + 1) * N_PER_CORE]),
            "r2t": r2t,
            "qt": qt,
        }
        for i in range(N_CORES)
    ]
    res = run_bass_kernel_spmd(nc, in_maps, list(range(N_CORES)))
    out = np.concatenate([res.results[i]["y"] for i in range(N_CORES)], axis=0)
    return out


# revision 5
# speedup vs baseline: 1.5958x; 1.5958x over previous
# Trainium2 Bass kernel for nn_Conv2dSDK_QR: low-rank (Q @ R) factorized
# stride-1 3x3 conv expressed as two matmuls over 4x4/stride-2 windows.
#
# Math (per image, validated vs reference):
#   xp = zero-pad(x, 1)                              [128, 66, 66]
#   flatT[win*128+c, vi*32+vj] = xp[c, i+2vi, j+2vj] (win = i*4+j)
#   tT = R2 @ flatT                                  [256, 1024]
#   yT = Q @ tT                                      [512, 1024]
#   out[oc, 2vi+top, 2vj+left] = yT[(top*2+left)*128+oc, vi*32+vj]
# where R2 is R with columns permuted from (c*16+win) to (win*128+c)
# ordering, so each win-chunk of flatT is just a strided view of xp.
#
# Device layouts (host pre/post-processed for contiguous PE streams):
#   x2[c, pi, pj, hi, wi] = xp[c, 2hi+pi, 2wi+pj]    (space-to-depth, 66=2x33)
#     -> window (i, j) l-block lb is the contiguous-inner view
#        x2[:, i&1, j&1, (i>>1)+16lb : +16, (j>>1) : +32]
#   y2[oc, top, left, vi, vj] = out[oc, 2vi+top, 2vj+left]  (parity planes)
#
# Sharding: data-parallel over batch, 4 images per core across 8 cores.

import numpy as np

import concourse.bacc as bacc
import concourse.bass as bass
import concourse.mybir as mybir
import concourse.tile as tile
from concourse.bass_utils import run_bass_kernel_spmd

N_CORES = 8
N_PER_CORE = 4
C = 128          # channels (= partition dim)
H = W = 64
HH = 33          # padded spatial per parity plane (66 = 2*33)
RANK = 256
MOUT = 512       # 4 placements * 128 out channels
NWIN = 16        # 4x4 window positions
DT = mybir.dt.float32
MM_DT = mybir.dt.float32r


def build_nc(n_per_core=N_PER_CORE, mm_dtype=MM_DT):
    nc = bacc.Bacc()
    x_ext = nc.declare_dram_parameter("x", [n_per_core, C, 2, 2, HH, HH], mm_dtype, isOutput=False)
    r_ext = nc.declare_dram_parameter("r2t", [C, NWIN, RANK], mm_dtype, isOutput=False)
    q_ext = nc.declare_dram_parameter("qt", [C, 2, MOUT], mm_dtype, isOutput=False)
    y_ext = nc.declare_dram_parameter("y", [n_per_core, C, 2, 2, 32, 32], DT, isOutput=True)

    with tile.TileContext(nc) as tc:
        with (
            tc.tile_pool(name="weights", bufs=1) as wpool,
            tc.tile_pool(name="xp", bufs=3) as xpool,
            tc.tile_pool(name="tt", bufs=2) as tpool,
            tc.tile_pool(name="osb", bufs=2) as opool,
            tc.tile_pool(name="pt", bufs=4, space="PSUM") as ptpool,
            tc.tile_pool(name="py", bufs=4, space="PSUM") as pypool,
        ):
            # r2t[c, win, r] = R2[r, win*128+c]  (lhsT chunks for matmul 1)
            r2t = wpool.tile([C, NWIN, RANK], mm_dtype)
            # qt[r_in, rc, m] = Q[m, rc*128+r_in]  (lhsT chunks for matmul 2)
            qt = wpool.tile([C, 2, MOUT], mm_dtype)
            nc.scalar.dma_start(r2t[:], r_ext[:])
            nc.scalar.dma_start(qt[:], q_ext[:])

            for n in range(n_per_core):
                x2 = xpool.tile([C, 2, 2, HH, HH], mm_dtype)
                nc.sync.dma_start(x2[:], x_ext[n])
                # tT[r_in, rc, lb, vi_in, vj] = tT_full[rc*128+r_in, (16*lb+vi_in)*32+vj]
                tT = tpool.tile([C, 2, 2, 16, 32], mm_dtype)
                osb = opool.tile([C, 2, 2, 32, 32], DT)
                for lb in range(2):   # l-blocks of 512 positions (16 vi rows)
                    for rc in range(2):   # rank tiles of 128
                        pt = ptpool.tile([128, 16, 32], DT)
                        for win in range(NWIN):
                            i, j = divmod(win, 4)
                            rhs = x2[:, i & 1, j & 1,
                                     (i >> 1) + 16 * lb : (i >> 1) + 16 * lb + 16,
                                     (j >> 1) : (j >> 1) + 32]
                            nc.tensor.matmul(
                                pt[:],
                                r2t[:, win, rc * 128 : (rc + 1) * 128],
                                rhs,
                                start=(win == 0),
                                stop=(win == NWIN - 1),
                            )
                        nc.vector.tensor_copy(tT[:, rc, lb], pt[:])
                    for mt in range(4):   # output row tiles: m = mt*128 + oc
                        py = pypool.tile([128, 16, 32], DT)
                        for rc in range(2):
                            nc.tensor.matmul(
                                py[:],
                                qt[:, rc, mt * 128 : (mt + 1) * 128],
                                tT[:, rc, lb],
                                start=(rc == 0),
                                stop=(rc == 1),
                            )
                        top, left = divmod(mt, 2)
                        nc.vector.tensor_copy(
                            osb[:, top, left, 16 * lb : 16 * lb + 16, :], py[:]
                        )
                nc.scalar.dma_start(y_ext[n], osb[:])
    nc.finalize()
    return nc


def make_host_inputs(x, Q, R, np_dtype=np.float32):
    """Full inputs -> (space-to-depth padded x, r2t, qt) host arrays."""
    x = np.asarray(x, dtype=np.float32)
    Q = np.asarray(Q, dtype=np.float32)
    R = np.asarray(R, dtype=np.float32)
    n = x.shape[0]
    xpad = np.zeros((n, C, 2 * HH, 2 * HH), np.float32)
    xpad[:, :, 1 : 1 + H, 1 : 1 + W] = x
    # space-to-depth: x2[n, c, pi, pj, hi, wi] = xpad[n, c, 2hi+pi, 2wi+pj]
    x2 = xpad.reshape(n, C, HH, 2, HH, 2).transpose(0, 1, 3, 5, 2, 4)
    x2 = np.ascontiguousarray(x2).astype(np_dtype)
    # permute R columns from (c*16+win) to (win*128+c)
    R2 = R.reshape(RANK, C, NWIN).transpose(0, 2, 1).reshape(RANK, C * NWIN)
    r2t = np.ascontiguousarray(R2.reshape(RANK, NWIN, C).transpose(2, 1, 0)).astype(np_dtype)
    qt = np.ascontiguousarray(Q.reshape(MOUT, 2, 128).transpose(2, 1, 0)).astype(np_dtype)
    return x2, r2t, qt


def unshard_output(ys):
    """Per-core [npc, C, 2, 2, 32, 32] parity planes -> [N, C, 64, 64]."""
    y2 = np.concatenate(ys, axis=0)
    n = y2.shape[0]
    y = y2.transpose(0, 1, 4, 2, 5, 3).reshape(n, C, 64, 64)
    return np.ascontiguousarray(y)


_NC_CACHE = {}


def kernel(x, Q, R):
    x2, r2t, qt = make_host_inputs(x, Q, R)
    n = x2.shape[0]
    assert n == N_CORES * N_PER_CORE
    if "nc" not in _NC_CACHE:
        _NC_CACHE["nc"] = build_nc()
    nc = _NC_CACHE["nc"]
    in_maps = [
        {
            "x": np.ascontiguousarray(x2[i * N_PER_CORE : (i + 1) * N_PER_CORE]),
            "r2t": r2t,
            "qt": qt,
        }
        for i in range(N_CORES)
    ]
    res = run_bass_kernel_spmd(nc, in_maps, list(range(N_CORES)))
    return unshard_output([res.results[i]["y"] for i in range(N_CORES)])
